# revision 33
# baseline (speedup 1.0000x reference)
"""Trainium2 Bass kernel for nn_FeatureContraction.

Computes out[b,c,w,x,v] = sum_i x[b,c,w,x,v,i] * node_attributes[b,c,i]
with B=C=128, X=3, Y=16 (wxv = 3*16*16 = 768, i = 16).

Strategy (8 NeuronCores, data-parallel over b; PE does the math):
  - x is uploaded as fp8 e3m4 (4 mantissa bits), host-packed so each
    b-slice is one [128, 12288] image: partition p = (c32, i4) with
    c32 = c%32 within a 32-channel group, i4 = i%4 within an i-chunk;
    free axis = (g, k, w).
  - per (g, k): one matmul with a block-diagonal stationary
    S[(c32,i4), c32'] = delta * na[32g+c32, 4k+i4] and moving rhs
    x[(c32,i4), w]; the 4 i-chunks (k) accumulate in PSUM; output
    strip = psum partitions [32g, 32g+32).
    Group 3 (strip base 96) is inexpressible as an AP base partition
    (rust IR allows only 0/32/64), so it uses a [128, 64] stationary
    [0 | diag] at base 64 and is emitted FIRST: its start=True zeroes
    rows 64-95, which group 2's own start=True then overwrites.
  - stationaries are built ON DEVICE by the (otherwise idle) DVE:
    S = mask * na_col from a host-packed f32 table (one DMA with the
    masks); the 3 narrow groups of one k-chunk build in a single
    broadcast tensor_tensor.  The 0.25 output pre-scale is folded into
    the table on host (undone by *4 on host after gather).
  - whole contraction per b-slice = 32 matmuls into one [128, 768]
    f32 PSUM image; ACT/DVE cast it to fp8 e3m4 (output rides at
    1 B/elem; rel err ~1.9e-2 vs the 2e-2 gate).
  - DMA plan (all x on the sync HWDGE ring -- cross-ring DMAs are
    globally issue-serialized and sem-lane coupled, so a second ring
    does not help): consts first, slice 0 as g3 per-k chunks then
    quarters (fast PE ramp), middle slices as g3-quarter + one
    3-quarter DMA (PE consumes g3 first, so it starts each slice
    ~3 us before the slice's tail lands), slice 15 as quarters with
    g2 per-k chunks last; slice 3 is contracted on the DVE
    (broadcast multiply + segment reduce) to keep the PE strictly
    under the stream pace.  Output stores ride the scalar HWDGE
    ring 4 slices per store; slice 15 drains/stores per 32-row
    strip so only g2's strip trails the last x byte.  A handful of
    zero matmuls at body start warm the PE HAM clock gate.
  - HBM per core: 24 MiB x + 1.5 MiB out (both fp8) -> ~75 us at
    358 GB/s; PE moving-data ~76 us busy -> both near-floor.
"""

import os
import sys

for _p in ("/opt/trn_rl_repo",):
    if _p not in sys.path:
        sys.path.append(_p)

import ml_dtypes
import numpy as np

import concourse.bass as bass
import concourse.mybir as mybir
import concourse.tile as tile
from concourse import bacc
from concourse.bass_utils import run_bass_kernel_spmd

# Problem dims (hardcoded per spec)
B, C, X, Y = 128, 128, 3, 16
WXV = X * Y * Y          # 768
I = Y                    # 16 (contraction axis)
N_CORES = 8
B_LOC = B // N_CORES     # 16 b-slices per core

NG = 4                   # channel groups of 32 (PSUM col-strip aligned)
CG = C // NG             # 32 channels per group
NK = 4                   # i-chunks of 4: K = CG*4 = 128 partitions
IK = I // NK             # 4
W_H0 = 512               # h0 span: exactly one 2 KB f32 PSUM bank
W_H1 = WXV - W_H0        # 256: the last-drained half, kept small for the tail
GQ = NK * WXV            # 3072: one group's x columns per b-slice
SKB = 2 * CG + 3 * CG    # 160 stationary cols per (b, k): [g3w|g0|g1|g2]
OBATCH = 4               # b-slices per output store
NACW = B_LOC * NK * NG   # na table cols per core: (b, k, g) g-contiguous
MSKW = 3 * CG            # mask cols: [mask64 | mask32]

F32 = mybir.dt.float32
BF16 = mybir.dt.bfloat16
F8E3 = mybir.dt.float8e3

X_DT = os.environ.get("FC_X_DT", "f8e3")  # "f8e3" | "bf16" for A/B tests
X_MYBIR_DT = {"f8e3": F8E3, "bf16": BF16}[X_DT]
X_NP_DT = {"f8e3": ml_dtypes.float8_e3m4, "bf16": ml_dtypes.bfloat16}[X_DT]

OUT_DT = os.environ.get("FC_OUT_DT", "f8e3")  # "f8e3" | "bf16"
OUT_MYBIR_DT = {"f8e3": F8E3, "bf16": BF16}[OUT_DT]
OUT_NP_DT = {"f8e3": ml_dtypes.float8_e3m4, "bf16": ml_dtypes.bfloat16}[OUT_DT]
# pre-scale folded into the na table so fp8 outputs stay in e3m4 range
OUT_PRESCALE = 0.25 if OUT_DT == "f8e3" else 1.0

GORDER = (3, 0, 1, 2)    # wide group 3 first (see module docstring)
N_WARM = 5               # zero matmuls to lift the PE HAM clock gate
OFFLOAD = (4,)           # DVE slice: early x arrival, but in batch 4-7
                         # whose store is naturally late (the DVE chain
                         # ends ~56us; batch 0-3 must not wait on it)
NA2W = len(OFFLOAD) * I  # na2 table cols for the DVE slices only

_COMPILED = None


def _build():
    nc = bacc.Bacc("TRN2", target_bir_lowering=False, debug=False,
                   num_devices=N_CORES)

    x_d = nc.dram_tensor("x", [B_LOC, 128, NG * GQ], X_MYBIR_DT,
                         kind="ExternalInput")
    # consts: [ nacol f32 (as 2 bf16 cols each) | mask64 | mask32 | na2 ]
    # in one bf16 tile; nacol is bitcast back to f32 (tensor_scalar
    # needs an f32 scalar), masks are exact in bf16
    cst_d = nc.dram_tensor("cst", [128, 2 * NACW + MSKW + NA2W], BF16,
                           kind="ExternalInput")
    # c-major so a multi-slice store has one contiguous line per partition
    out_d = nc.dram_tensor("out", [C, B_LOC, WXV], OUT_MYBIR_DT,
                           kind="ExternalOutput")

    with tile.TileContext(nc) as tc:
        with (
            tc.tile_pool(name="const", bufs=1) as constp,
            tc.tile_pool(name="xp", bufs=5) as xp,
            tc.tile_pool(name="xn", bufs=1) as xnp,
            tc.tile_pool(name="xq", bufs=6) as xqp,
            tc.tile_pool(name="sp", bufs=B_LOC - len(OFFLOAD)) as sp,
            tc.tile_pool(name="yp", bufs=1) as yp,
            tc.tile_pool(name="yr", bufs=1) as yrp,
            tc.tile_pool(name="outp", bufs=3) as outp,
            tc.tile_pool(name="psp", bufs=4, space="PSUM") as psp,
        ):
            cst = constp.tile([128, 2 * NACW + MSKW + NA2W], BF16)
            nacol = cst[:, : 2 * NACW].bitcast(F32)
            m64 = cst[:, 2 * NACW : 2 * NACW + 2 * CG]
            m32 = cst[:, 2 * NACW + 2 * CG : 2 * NACW + MSKW]
            na2 = cst[:, 2 * NACW + MSKW :] if OFFLOAD else None

            # PE warm-up: zero matmuls on a memset tile lift the HAM
            # clock gate (1.2 -> 2.4 GHz takes ~3.4 us of PE activity)
            # while the first x tile is still in flight
            wt = constp.tile([128, 32 + W_H0], BF16)
            nc.gpsimd.memset(wt[:], 0)
            dps = psp.tile([128, W_H0], F32, tag="ps0")
            for _ in range(N_WARM):
                nc.tensor.matmul(dps[:32, :], wt[:, :32], wt[:, 32:],
                                 start=True, stop=True)

            # x load schedule, all on the sync HWDGE FIFO, emitted up
            # front: small chunks at the ramp (PE starts on 98 KB), one
            # 1.57 MB line-rate DMA per slice in the steady state,
            # chunks again at the tail so the last bytes are consumed
            # on arrival.  Buffer recycling paces the later issues.
            xloads = {}

            def load_chunks(b, g):
                xck = []
                for kk in range(NK):
                    xc = xqp.tile([128, WXV], X_MYBIR_DT, tag="xk")
                    nc.sync.dma_start(
                        xc[:],
                        x_d[b, :, g * GQ + kk * WXV : g * GQ + (kk + 1) * WXV])
                    xck.append(xc)
                xloads.setdefault(b, {})[g] = xck

            def load_quarter(b, g):
                xt = xqp.tile([128, GQ], X_MYBIR_DT, tag="x4")
                nc.sync.dma_start(xt[:], x_d[b, :, g * GQ : (g + 1) * GQ])
                xloads.setdefault(b, {})[g] = xt

            def load_full(b):
                if b in OFFLOAD:
                    xf = xnp.tile([128, NG * GQ], X_MYBIR_DT, tag="x")
                    nc.sync.dma_start(xf[:], x_d[b, :, :])
                    xloads.setdefault(b, {})["nat"] = xf
                    return
                # g3's quarter rides FIRST (PE consumes g3 first), the
                # other three follow in one DMA: PE starts each slice
                # ~3 us earlier than with a single whole-slice DMA
                xa = xp.tile([128, GQ], X_MYBIR_DT, tag="xa")
                nc.sync.dma_start(xa[:], x_d[b, :, 3 * GQ : 4 * GQ])
                xr = xp.tile([128, 3 * GQ], X_MYBIR_DT, tag="xr")
                nc.sync.dma_start(xr[:], x_d[b, :, 0 : 3 * GQ])
                xloads.setdefault(b, {})[3] = xa
                for g in range(3):
                    xloads[b][g] = xr[:, g * GQ : (g + 1) * GQ]

            # slice 0's g3 chunks lead the sync FIFO so the x stream
            # (the pacer) starts ~0.7 us earlier; the const load rides
            # just behind them -- the PE's ramp slack absorbs the
            # slightly later stationary builds
            load_chunks(0, 3)
            nc.sync.dma_start(cst[:], cst_d[:])
            for g in (0, 1, 2):        # slice 0 quarters
                load_quarter(0, g)
            for b in range(1, B_LOC - 1):
                load_full(b)
            # slice 15: quarters g3,g0,g1 (each strip's matmuls and
            # drains start on its own arrival), g2 per-k chunks last
            # (consumed on arrival at the stream tail)
            load_quarter(15, 3)
            load_quarter(15, 0)
            load_quarter(15, 1)
            load_chunks(15, 2)

            # all stationaries build upfront on DVE (they only need the
            # const tile); DVE's later offload-slice work then never
            # blocks a build the PE is waiting for
            sts = {}
            for b in range(B_LOC):
                if b in OFFLOAD:
                    continue
                st = sp.tile([128, NK * SKB], BF16, tag="s")
                sts[b] = st
                for k in range(NK):
                    j = (b * NK + k) * NG
                    nc.vector.tensor_scalar_mul(
                        st[:, k * SKB : k * SKB + 2 * CG],
                        m64, nacol[:, j + 3 : j + 4])
                    nc.vector.tensor_tensor(
                        st[:, k * SKB + 2 * CG : (k + 1) * SKB]
                        .rearrange("p (g c) -> p g c", g=3),
                        nacol[:, j : j + 3].unsqueeze(-1)
                        .broadcast_to([128, 3, CG]),
                        m32.unsqueeze(1).broadcast_to([128, 3, CG]),
                        mybir.AluOpType.mult)

            ot = None
            ota = None
            for b in range(B_LOC):
                xts = xloads.pop(b)
                first_b = b == 0
                last_b = b == B_LOC - 1

                if b in OFFLOAD:
                    # DVE path: x for this slice is packed [c, (w, i)]
                    # (natural layout); broadcast-multiply by na2[c, i]
                    # then segment-reduce the 16 i's; ACT casts to fp8
                    xn = xts["nat"]
                    y = yp.tile([128, NG * GQ], BF16, tag="y")
                    nc.vector.tensor_tensor(
                        y[:].rearrange("p (w i) -> p w i", i=I),
                        xn[:].rearrange("p (w i) -> p w i", i=I),
                        na2[:, OFFLOAD.index(b) * I :
                            (OFFLOAD.index(b) + 1) * I].unsqueeze(1)
                        .broadcast_to([128, WXV, I]),
                        mybir.AluOpType.mult)
                    red = yrp.tile([128, WXV], F32, tag="r")
                    nc.vector.tensor_reduce(
                        red[:], y[:].rearrange("p (w i) -> p w i", i=I),
                        mybir.AxisListType.X, mybir.AluOpType.add)
                    ob = b % OBATCH
                    if ob == 0:
                        ot = outp.tile([C, OBATCH * WXV], OUT_MYBIR_DT,
                                       tag="out")
                    o0 = ob * WXV
                    nc.scalar.copy(ot[:, o0 : o0 + WXV], red[:])
                    if ob == OBATCH - 1:
                        b0 = b - (OBATCH - 1)
                        nc.scalar.dma_start(out_d[:, b0 : b0 + OBATCH, :],
                                            ot[:])
                    continue

                st = sts[b]
                ps0 = psp.tile([128, W_H0], F32, tag="ps0")
                ps1 = psp.tile([128, W_H1], F32, tag="ps1")
                ps = {0: ps0, 1: ps1}
                hspan = {0: (0, W_H0), 1: (W_H0, WXV)}
                otb = None
                if last_b:
                    otb = outp.tile([C, WXV], OUT_MYBIR_DT, tag="outb")
                for g in GORDER:
                    if last_b and g == 2:
                        # strips g0+g1 (psum rows 0:64) and g3 (96:128)
                        # are complete: drain + store them NOW, before
                        # g2's matmuls are emitted, so their waits don't
                        # inherit g2's later ticks and they fly while
                        # the stream tail lands
                        nc.scalar.copy(otb[0:64, :W_H0], ps[0][0:64, :])
                        nc.vector.tensor_copy(otb[0:64, W_H0:],
                                              ps[1][0:64, :])
                        nc.scalar.copy(otb[96:128, :W_H0],
                                       ps[0][96:128, :])
                        nc.vector.tensor_copy(otb[96:128, W_H0:],
                                              ps[1][96:128, :])
                        nc.sync.dma_start(out_d[0:64, b : b + 1, :],
                                          otb[0:64, :])
                        nc.sync.dma_start(out_d[96:128, b : b + 1, :],
                                          otb[96:128, :])
                    if (first_b and g == 3) or (last_b and g == 2):
                        # chunked tail/head group: k-inner per w-region
                        # (j outer) so each PSUM bank has at most ONE
                        # open accumulation group at a time (interleaved
                        # groups within a bank lose accumulations on HW)
                        nck = len(xts[g])
                        kh = nck // NK  # chunks per k
                        for j in range(kh):
                            c0 = j * (WXV // kh)
                            for k in range(NK):
                                if g == 3:
                                    lhsT = st[:, k * SKB : k * SKB + 2 * CG]
                                    oap = {h: ps[h][2 * CG : 4 * CG, :]
                                           for h in range(2)}
                                else:
                                    s0 = k * SKB + 2 * CG + g * CG
                                    lhsT = st[:, s0 : s0 + CG]
                                    oap = {h: ps[h][CG * g : CG * (g + 1), :]
                                           for h in range(2)}
                                for h in range(2):
                                    w0, w1 = hspan[h]
                                    lo = max(w0, c0)
                                    hi = min(w1, c0 + WXV // kh)
                                    if lo >= hi:
                                        continue
                                    nc.tensor.matmul(
                                        oap[h][:, lo - w0 : hi - w0],
                                        lhsT,
                                        xts[g][k * kh + j][:, lo - c0 : hi - c0],
                                        start=(k == 0),
                                        stop=(k == NK - 1),
                                    )
                        continue
                    for h in range(2):
                        w0, w1 = hspan[h]
                        for k in range(NK):
                            if g == 3:
                                lhsT = st[:, k * SKB : k * SKB + 2 * CG]
                                oap = ps[h][2 * CG : 4 * CG, :]
                            else:
                                s0 = k * SKB + 2 * CG + g * CG
                                lhsT = st[:, s0 : s0 + CG]
                                oap = ps[h][CG * g : CG * (g + 1), :]
                            nc.tensor.matmul(
                                oap,
                                lhsT,
                                xts[g][:, k * WXV + w0 : k * WXV + w1],
                                start=(k == 0),
                                stop=(k == NK - 1),
                            )

                # ACT drains both PSUM halves (DVE is busy with the
                # offload slices); stores ride the scalar HWDGE ring
                # except the last, which takes the (drained) sync ring
                if last_b:
                    # only g2's strip (64:96, fed by the last chunks)
                    # rides the critical tail; the rest stored above
                    nc.scalar.copy(otb[64:96, :W_H0], ps[0][64:96, :])
                    nc.vector.tensor_copy(otb[64:96, W_H0:],
                                          ps[1][64:96, :])
                    nc.sync.dma_start(out_d[64:96, b : b + 1, :],
                                      otb[64:96, :])
                elif b >= 12:
                    ob = b - 12
                    if ob == 0:
                        ota = outp.tile([C, 3 * WXV], OUT_MYBIR_DT, tag="outa")
                    o0 = ob * WXV
                    nc.scalar.copy(ota[:, o0 : o0 + W_H0], ps[0][:])
                    nc.scalar.copy(ota[:, o0 + W_H0 : o0 + WXV], ps[1][:])
                    if ob == 2:
                        nc.scalar.dma_start(out_d[:, 12:15, :], ota[:])
                else:
                    ob = b % OBATCH
                    if ob == 0:
                        ot = outp.tile([C, OBATCH * WXV], OUT_MYBIR_DT,
                                       tag="out")
                    o0 = ob * WXV
                    nc.scalar.copy(ot[:, o0 : o0 + W_H0], ps[0][:])
                    nc.scalar.copy(ot[:, o0 + W_H0 : o0 + WXV], ps[1][:])
                    if ob == OBATCH - 1:
                        b0 = b - (OBATCH - 1)
                        nc.scalar.dma_start(out_d[:, b0 : b0 + OBATCH, :],
                                            ot[:])

    nc.compile()
    return nc


def _get_compiled():
    global _COMPILED
    if _COMPILED is None:
        _COMPILED = _build()
    return _COMPILED


def _make_in_maps(inputs: dict):
    x = np.asarray(inputs["x"], dtype=np.float32)
    na = np.asarray(inputs["node_attributes"], dtype=np.float32)

    # x[b, c, w, i] -> xq[b, p=(c32,i4), (g, k), w], cast first (cheaper
    # to transpose 1-2 B elems than 4 B); the DVE-offload slices keep
    # the natural [c, (w, i)] layout instead
    x8 = x.reshape(B, C, WXV, I).astype(X_NP_DT)
    xq = x8.reshape(B, NG, CG, WXV, NK, IK)
    xq = np.ascontiguousarray(xq.transpose(0, 2, 5, 1, 4, 3))
    xq = xq.reshape(B, 128, NG * GQ)
    xnat = x8.reshape(B, C, WXV * I)
    for kcore in range(N_CORES):
        for bo in OFFLOAD:
            xq[kcore * B_LOC + bo] = xnat[kcore * B_LOC + bo]

    # na_col[p=(c32,i4), (b, k, g)] = na[b, 32g+c32, 4k+i4] * prescale
    nacol = na.reshape(B, NG, CG, NK, IK).transpose(2, 4, 0, 3, 1)
    nacol = np.ascontiguousarray(nacol).reshape(128, B * NK * NG)
    nacol = (nacol * OUT_PRESCALE).astype(np.float32)

    # masks: mask64[p, j] = (j >= 32) & (p//4 == j-32); mask32[p, m] = (p//4 == m)
    p4 = np.arange(128) // IK
    m32 = (p4[:, None] == np.arange(CG)[None, :])
    mask = np.concatenate(
        [np.zeros((128, CG), bool), m32, m32], axis=1
    ).astype(np.float32)

    # na2[c, (b, i)] = na[b, c, i] * prescale (for the DVE slices)
    na2 = (na.transpose(1, 0, 2) * OUT_PRESCALE).astype(np.float32)

    in_maps = []
    for kcore in range(N_CORES):
        b0 = kcore * B_LOC
        nci = nacol.reshape(128, B, NK * NG)[:, b0 : b0 + B_LOC]
        ncif = np.ascontiguousarray(nci).reshape(128, -1).astype(np.float32)
        parts = [ncif.view(ml_dtypes.bfloat16),
                 mask.astype(ml_dtypes.bfloat16)]
        if OFFLOAD:
            na2c = np.concatenate(
                [na2[:, b0 + bo] for bo in OFFLOAD], axis=1)
            parts.append(np.ascontiguousarray(na2c)
                         .astype(ml_dtypes.bfloat16))
        cst = np.ascontiguousarray(np.concatenate(parts, axis=1))
        in_maps.append(
            {
                "x": xq[b0 : b0 + B_LOC],
                "cst": np.ascontiguousarray(cst),
            }
        )
    return in_maps


def _gather(results) -> np.ndarray:
    # per-core out is [C, B_LOC, WXV] (c-major for store efficiency)
    out = np.concatenate(
        [np.asarray(r["out"]).transpose(1, 0, 2) for r in results], axis=0
    )
    out = out.astype(np.float32) * (1.0 / OUT_PRESCALE)
    return out.reshape(B, C, X, Y, Y)


def _run(inputs: dict, trace: bool = False, trace_cores=None):
    in_maps = _make_in_maps(inputs)
    nc = _get_compiled()
    res = run_bass_kernel_spmd(
        nc,
        in_maps,
        core_ids=list(range(N_CORES)),
        trace=trace,
        trace_cores=trace_cores,
    )
    return _gather(res.results), res


def kernel(**inputs) -> np.ndarray:
    out, _ = _run(inputs, trace=False)
    return out


# revision 36
# speedup vs baseline: 1.0103x; 1.0103x over previous
"""Trainium2 Bass kernel for nn_FeatureContraction.

Computes out[b,c,w,x,v] = sum_i x[b,c,w,x,v,i] * node_attributes[b,c,i]
with B=C=128, X=3, Y=16 (wxv = 3*16*16 = 768, i = 16).

Strategy (8 NeuronCores, data-parallel over b; PE does the math):
  - x is uploaded as fp8 e3m4 (4 mantissa bits), host-packed so each
    b-slice is one [128, 12288] image: partition p = (c32, i4) with
    c32 = c%32 within a 32-channel group, i4 = i%4 within an i-chunk;
    free axis = (g, k, w).
  - per (g, k): one matmul with a block-diagonal stationary
    S[(c32,i4), c32'] = delta * na[32g+c32, 4k+i4] and moving rhs
    x[(c32,i4), w]; the 4 i-chunks (k) accumulate in PSUM; output
    strip = psum partitions [32g, 32g+32).
    Group 3 (strip base 96) is inexpressible as an AP base partition
    (rust IR allows only 0/32/64), so it uses a [128, 64] stationary
    [0 | diag] at base 64 and is emitted FIRST: its start=True zeroes
    rows 64-95, which group 2's own start=True then overwrites.
  - stationaries are built ON DEVICE by the (otherwise idle) DVE:
    S = mask * na_col from a host-packed f32 table (one DMA with the
    masks); the 3 narrow groups of one k-chunk build in a single
    broadcast tensor_tensor.  The 0.25 output pre-scale is folded into
    the table on host (undone by *4 on host after gather).
  - whole contraction per b-slice = 32 matmuls into one [128, 768]
    f32 PSUM image; ACT/DVE cast it to fp8 e3m4 (output rides at
    1 B/elem; rel err ~1.9e-2 vs the 2e-2 gate).
  - DMA plan (all x on the sync HWDGE ring -- cross-ring DMAs are
    globally issue-serialized and sem-lane coupled, so a second ring
    does not help): consts first, slice 0 as g3 per-k chunks then
    quarters (fast PE ramp), middle slices as g3-quarter + one
    3-quarter DMA (PE consumes g3 first, so it starts each slice
    ~3 us before the slice's tail lands), slice 15 as quarters with
    g2 per-k chunks last; slice 3 is contracted on the DVE
    (broadcast multiply + segment reduce) to keep the PE strictly
    under the stream pace.  Output stores ride the scalar HWDGE
    ring 4 slices per store; slice 15 drains/stores per 32-row
    strip so only g2's strip trails the last x byte.  A handful of
    zero matmuls at body start warm the PE HAM clock gate.
  - HBM per core: 24 MiB x + 1.5 MiB out (both fp8) -> ~75 us at
    358 GB/s; PE moving-data ~76 us busy -> both near-floor.
"""

import os
import sys

for _p in ("/opt/trn_rl_repo",):
    if _p not in sys.path:
        sys.path.append(_p)

import ml_dtypes
import numpy as np

import concourse.bass as bass
import concourse.mybir as mybir
import concourse.tile as tile
from concourse import bacc
from concourse.bass_utils import run_bass_kernel_spmd

# Problem dims (hardcoded per spec)
B, C, X, Y = 128, 128, 3, 16
WXV = X * Y * Y          # 768
I = Y                    # 16 (contraction axis)
N_CORES = 8
B_LOC = B // N_CORES     # 16 b-slices per core

NG = 4                   # channel groups of 32 (PSUM col-strip aligned)
CG = C // NG             # 32 channels per group
NK = 4                   # i-chunks of 4: K = CG*4 = 128 partitions
IK = I // NK             # 4
W_H0 = 512               # h0 span: exactly one 2 KB f32 PSUM bank
W_H1 = WXV - W_H0        # 256: the last-drained half, kept small for the tail
GQ = NK * WXV            # 3072: one group's x columns per b-slice
SKB = 2 * CG + 3 * CG    # 160 stationary cols per (b, k): [g3w|g0|g1|g2]
OBATCH = 4               # b-slices per output store
NACW = B_LOC * NK * NG   # na table cols per core: (b, k, g) g-contiguous
MSKW = 3 * CG            # mask cols: [mask64 | mask32]

F32 = mybir.dt.float32
BF16 = mybir.dt.bfloat16
F8E3 = mybir.dt.float8e3

X_DT = os.environ.get("FC_X_DT", "f8e3")  # "f8e3" | "bf16" for A/B tests
X_MYBIR_DT = {"f8e3": F8E3, "bf16": BF16}[X_DT]
X_NP_DT = {"f8e3": ml_dtypes.float8_e3m4, "bf16": ml_dtypes.bfloat16}[X_DT]

OUT_DT = os.environ.get("FC_OUT_DT", "f8e3")  # "f8e3" | "bf16"
OUT_MYBIR_DT = {"f8e3": F8E3, "bf16": BF16}[OUT_DT]
OUT_NP_DT = {"f8e3": ml_dtypes.float8_e3m4, "bf16": ml_dtypes.bfloat16}[OUT_DT]
# pre-scale folded into the na table so fp8 outputs stay in e3m4 range
OUT_PRESCALE = 0.25 if OUT_DT == "f8e3" else 1.0

GORDER = (3, 0, 1, 2)    # wide group 3 first (see module docstring)
N_WARM = 5               # zero matmuls to lift the PE HAM clock gate
OFFLOAD = (4,)           # DVE slice: early x arrival, but in batch 4-7
                         # whose store is naturally late (the DVE chain
                         # ends ~56us; batch 0-3 must not wait on it)
NA2W = len(OFFLOAD) * I  # na2 table cols for the DVE slices only

_COMPILED = None


def _build():
    nc = bacc.Bacc("TRN2", target_bir_lowering=False, debug=False,
                   num_devices=N_CORES)

    # one input image per core: [ cst | slice0 g3k0 | slice0 g3k1-3 |
    # slice0 g0-2 | slices 1..15 ] so the stream's FIRST dma carries
    # both the consts and the first PE chunk in a single issue slot.
    # cst bytes: nacol as f32, masks/na2 as bf16, all viewed as fp8.
    CSTF8 = 4 * NACW + 2 * (MSKW + NA2W)
    SL0 = CSTF8 + NG * GQ
    x_d = nc.dram_tensor("x", [128, SL0 + (B_LOC - 1) * NG * GQ],
                         X_MYBIR_DT, kind="ExternalInput")
    # c-major so a multi-slice store has one contiguous line per partition
    out_d = nc.dram_tensor("out", [C, B_LOC, WXV], OUT_MYBIR_DT,
                           kind="ExternalOutput")

    with tile.TileContext(nc) as tc:
        with (
            tc.tile_pool(name="const", bufs=1) as constp,
            tc.tile_pool(name="xp", bufs=5) as xp,
            tc.tile_pool(name="xn", bufs=1) as xnp,
            tc.tile_pool(name="xq", bufs=6) as xqp,
            tc.tile_pool(name="sp", bufs=B_LOC - len(OFFLOAD)) as sp,
            tc.tile_pool(name="yp", bufs=1) as yp,
            tc.tile_pool(name="yr", bufs=1) as yrp,
            tc.tile_pool(name="outp", bufs=3) as outp,
            tc.tile_pool(name="psp", bufs=4, space="PSUM") as psp,
        ):
            # head tile = consts + slice0's g3k0 chunk (one DMA)
            hd = constp.tile([128, CSTF8 + WXV], X_MYBIR_DT)
            nacol = hd[:, : 4 * NACW].bitcast(F32)
            m64 = hd[:, 4 * NACW : 4 * NACW + 4 * CG].bitcast(BF16)
            m32 = hd[:, 4 * NACW + 4 * CG : 4 * NACW + 6 * CG].bitcast(BF16)
            na2 = hd[:, 4 * NACW + 6 * CG : CSTF8].bitcast(BF16)

            # PE warm-up: zero matmuls on a memset tile lift the HAM
            # clock gate (1.2 -> 2.4 GHz takes ~3.4 us of PE activity)
            # while the first x tile is still in flight
            wt = constp.tile([128, 32 + W_H0], BF16)
            nc.gpsimd.memset(wt[:], 0)
            dps = psp.tile([128, W_H0], F32, tag="ps0")
            for _ in range(N_WARM):
                nc.tensor.matmul(dps[:32, :], wt[:, :32], wt[:, 32:],
                                 start=True, stop=True)

            # x load schedule, all on the sync HWDGE FIFO, emitted up
            # front: small chunks at the ramp (PE starts on 98 KB), one
            # 1.57 MB line-rate DMA per slice in the steady state,
            # chunks again at the tail so the last bytes are consumed
            # on arrival.  Buffer recycling paces the later issues.
            xloads = {}

            def xoff(b, c0):
                # slice b's column c0 in the custom DRAM layout
                assert b >= 1
                return SL0 + (b - 1) * NG * GQ + c0

            def load_chunks(b, g):
                xck = []
                for kk in range(NK):
                    xc = xqp.tile([128, WXV], X_MYBIR_DT, tag="xk")
                    nc.sync.dma_start(
                        xc[:],
                        x_d[:, xoff(b, g * GQ + kk * WXV) :
                            xoff(b, g * GQ + (kk + 1) * WXV)])
                    xck.append(xc)
                xloads.setdefault(b, {})[g] = xck

            def load_quarter(b, g):
                xt = xqp.tile([128, GQ], X_MYBIR_DT, tag="x4")
                nc.sync.dma_start(
                    xt[:], x_d[:, xoff(b, g * GQ) : xoff(b, (g + 1) * GQ)])
                xloads.setdefault(b, {})[g] = xt

            def load_full(b):
                if b in OFFLOAD:
                    xf = xnp.tile([128, NG * GQ], X_MYBIR_DT, tag="x")
                    nc.sync.dma_start(
                        xf[:], x_d[:, xoff(b, 0) : xoff(b, NG * GQ)])
                    xloads.setdefault(b, {})["nat"] = xf
                    return
                # g3's quarter rides FIRST (PE consumes g3 first), the
                # other three follow in one DMA: PE starts each slice
                # ~3 us earlier than with a single whole-slice DMA
                xa = xp.tile([128, GQ], X_MYBIR_DT, tag="xa")
                nc.sync.dma_start(
                    xa[:], x_d[:, xoff(b, 3 * GQ) : xoff(b, 4 * GQ)])
                xr = xp.tile([128, 3 * GQ], X_MYBIR_DT, tag="xr")
                nc.sync.dma_start(xr[:], x_d[:, xoff(b, 0) : xoff(b, 3 * GQ)])
                xloads.setdefault(b, {})[3] = xa
                for g in range(3):
                    xloads[b][g] = xr[:, g * GQ : (g + 1) * GQ]

            # head DMA: consts + slice0 g3k0 in ONE issue slot leads
            # the sync FIFO; g3 k1-3 chunks and the quarters follow
            # (custom DRAM layout puts them contiguously after cst)
            nc.sync.dma_start(hd[:], x_d[:, : CSTF8 + WXV])
            xck0 = [hd[:, CSTF8:]]
            for kk in range(1, NK):
                xc = xqp.tile([128, WXV], X_MYBIR_DT, tag="xk")
                nc.sync.dma_start(
                    xc[:], x_d[:, CSTF8 + kk * WXV : CSTF8 + (kk + 1) * WXV])
                xck0.append(xc)
            xloads[0] = {3: xck0}
            for g in (0, 1, 2):        # slice 0 quarters
                xt = xqp.tile([128, GQ], X_MYBIR_DT, tag="x4")
                nc.sync.dma_start(
                    xt[:], x_d[:, CSTF8 + NK * WXV + g * GQ :
                               CSTF8 + NK * WXV + (g + 1) * GQ])
                xloads[0][g] = xt
            for b in range(1, B_LOC - 1):
                load_full(b)
            # slice 15: quarters g3,g0,g1 (each strip's matmuls and
            # drains start on its own arrival), g2 per-k chunks last
            # (consumed on arrival at the stream tail)
            load_quarter(15, 3)
            load_quarter(15, 0)
            load_quarter(15, 1)
            load_chunks(15, 2)

            # all stationaries build upfront on DVE (they only need the
            # const tile); DVE's later offload-slice work then never
            # blocks a build the PE is waiting for
            sts = {}
            for b in range(B_LOC):
                if b in OFFLOAD:
                    continue
                st = sp.tile([128, NK * SKB], BF16, tag="s")
                sts[b] = st
                for k in range(NK):
                    j = (b * NK + k) * NG
                    nc.vector.tensor_scalar_mul(
                        st[:, k * SKB : k * SKB + 2 * CG],
                        m64, nacol[:, j + 3 : j + 4])
                    nc.vector.tensor_tensor(
                        st[:, k * SKB + 2 * CG : (k + 1) * SKB]
                        .rearrange("p (g c) -> p g c", g=3),
                        nacol[:, j : j + 3].unsqueeze(-1)
                        .broadcast_to([128, 3, CG]),
                        m32.unsqueeze(1).broadcast_to([128, 3, CG]),
                        mybir.AluOpType.mult)

            ot = None
            ota = None
            for b in range(B_LOC):
                xts = xloads.pop(b)
                first_b = b == 0
                last_b = b == B_LOC - 1

                if b in OFFLOAD:
                    # DVE path: x for this slice is packed [c, (w, i)]
                    # (natural layout); broadcast-multiply by na2[c, i]
                    # then segment-reduce the 16 i's; ACT casts to fp8
                    xn = xts["nat"]
                    y = yp.tile([128, NG * GQ], BF16, tag="y")
                    nc.vector.tensor_tensor(
                        y[:].rearrange("p (w i) -> p w i", i=I),
                        xn[:].rearrange("p (w i) -> p w i", i=I),
                        na2[:, OFFLOAD.index(b) * I :
                            (OFFLOAD.index(b) + 1) * I].unsqueeze(1)
                        .broadcast_to([128, WXV, I]),
                        mybir.AluOpType.mult)
                    red = yrp.tile([128, WXV], F32, tag="r")
                    nc.vector.tensor_reduce(
                        red[:], y[:].rearrange("p (w i) -> p w i", i=I),
                        mybir.AxisListType.X, mybir.AluOpType.add)
                    ob = b % OBATCH
                    if ob == 0:
                        ot = outp.tile([C, OBATCH * WXV], OUT_MYBIR_DT,
                                       tag="out")
                    o0 = ob * WXV
                    nc.scalar.copy(ot[:, o0 : o0 + WXV], red[:])
                    if ob == OBATCH - 1:
                        b0 = b - (OBATCH - 1)
                        nc.scalar.dma_start(out_d[:, b0 : b0 + OBATCH, :],
                                            ot[:])
                    continue

                st = sts[b]
                ps0 = psp.tile([128, W_H0], F32, tag="ps0")
                ps1 = psp.tile([128, W_H1], F32, tag="ps1")
                ps = {0: ps0, 1: ps1}
                hspan = {0: (0, W_H0), 1: (W_H0, WXV)}
                otb = None
                if last_b:
                    otb = outp.tile([C, WXV], OUT_MYBIR_DT, tag="outb")
                for g in GORDER:
                    if last_b and g == 2:
                        # strips g0+g1 (psum rows 0:64) and g3 (96:128)
                        # are complete: drain + store them NOW, before
                        # g2's matmuls are emitted, so their waits don't
                        # inherit g2's later ticks and they fly while
                        # the stream tail lands
                        nc.scalar.copy(otb[0:64, :W_H0], ps[0][0:64, :])
                        nc.vector.tensor_copy(otb[0:64, W_H0:],
                                              ps[1][0:64, :])
                        nc.scalar.copy(otb[96:128, :W_H0],
                                       ps[0][96:128, :])
                        nc.vector.tensor_copy(otb[96:128, W_H0:],
                                              ps[1][96:128, :])
                        nc.sync.dma_start(out_d[0:64, b : b + 1, :],
                                          otb[0:64, :])
                        nc.sync.dma_start(out_d[96:128, b : b + 1, :],
                                          otb[96:128, :])
                    if (first_b and g == 3) or (last_b and g == 2):
                        # chunked tail/head group: k-inner per w-region
                        # (j outer) so each PSUM bank has at most ONE
                        # open accumulation group at a time (interleaved
                        # groups within a bank lose accumulations on HW)
                        nck = len(xts[g])
                        kh = nck // NK  # chunks per k
                        for j in range(kh):
                            c0 = j * (WXV // kh)
                            for k in range(NK):
                                if g == 3:
                                    lhsT = st[:, k * SKB : k * SKB + 2 * CG]
                                    oap = {h: ps[h][2 * CG : 4 * CG, :]
                                           for h in range(2)}
                                else:
                                    s0 = k * SKB + 2 * CG + g * CG
                                    lhsT = st[:, s0 : s0 + CG]
                                    oap = {h: ps[h][CG * g : CG * (g + 1), :]
                                           for h in range(2)}
                                for h in range(2):
                                    w0, w1 = hspan[h]
                                    lo = max(w0, c0)
                                    hi = min(w1, c0 + WXV // kh)
                                    if lo >= hi:
                                        continue
                                    nc.tensor.matmul(
                                        oap[h][:, lo - w0 : hi - w0],
                                        lhsT,
                                        xts[g][k * kh + j][:, lo - c0 : hi - c0],
                                        start=(k == 0),
                                        stop=(k == NK - 1),
                                    )
                        continue
                    for h in range(2):
                        w0, w1 = hspan[h]
                        for k in range(NK):
                            if g == 3:
                                lhsT = st[:, k * SKB : k * SKB + 2 * CG]
                                oap = ps[h][2 * CG : 4 * CG, :]
                            else:
                                s0 = k * SKB + 2 * CG + g * CG
                                lhsT = st[:, s0 : s0 + CG]
                                oap = ps[h][CG * g : CG * (g + 1), :]
                            nc.tensor.matmul(
                                oap,
                                lhsT,
                                xts[g][:, k * WXV + w0 : k * WXV + w1],
                                start=(k == 0),
                                stop=(k == NK - 1),
                            )

                # ACT drains both PSUM halves (DVE is busy with the
                # offload slices); stores ride the scalar HWDGE ring
                # except the last, which takes the (drained) sync ring
                if last_b:
                    # only g2's strip (64:96, fed by the last chunks)
                    # rides the critical tail; the rest stored above
                    nc.scalar.copy(otb[64:96, :W_H0], ps[0][64:96, :])
                    nc.vector.tensor_copy(otb[64:96, W_H0:],
                                          ps[1][64:96, :])
                    nc.sync.dma_start(out_d[64:96, b : b + 1, :],
                                      otb[64:96, :])
                elif b >= 12:
                    ob = b - 12
                    if ob == 0:
                        ota = outp.tile([C, 3 * WXV], OUT_MYBIR_DT, tag="outa")
                    o0 = ob * WXV
                    nc.scalar.copy(ota[:, o0 : o0 + W_H0], ps[0][:])
                    nc.scalar.copy(ota[:, o0 + W_H0 : o0 + WXV], ps[1][:])
                    if ob == 2:
                        nc.scalar.dma_start(out_d[:, 12:15, :], ota[:])
                else:
                    ob = b % OBATCH
                    if ob == 0:
                        ot = outp.tile([C, OBATCH * WXV], OUT_MYBIR_DT,
                                       tag="out")
                    o0 = ob * WXV
                    nc.scalar.copy(ot[:, o0 : o0 + W_H0], ps[0][:])
                    nc.scalar.copy(ot[:, o0 + W_H0 : o0 + WXV], ps[1][:])
                    if ob == OBATCH - 1:
                        b0 = b - (OBATCH - 1)
                        nc.scalar.dma_start(out_d[:, b0 : b0 + OBATCH, :],
                                            ot[:])

    nc.compile()
    return nc


def _get_compiled():
    global _COMPILED
    if _COMPILED is None:
        _COMPILED = _build()
    return _COMPILED


def _make_in_maps(inputs: dict):
    x = np.asarray(inputs["x"], dtype=np.float32)
    na = np.asarray(inputs["node_attributes"], dtype=np.float32)

    # x[b, c, w, i] -> xq[b, p=(c32,i4), (g, k), w], cast first (cheaper
    # to transpose 1-2 B elems than 4 B); the DVE-offload slices keep
    # the natural [c, (w, i)] layout instead
    x8 = x.reshape(B, C, WXV, I).astype(X_NP_DT)
    xq = x8.reshape(B, NG, CG, WXV, NK, IK)
    xq = np.ascontiguousarray(xq.transpose(0, 2, 5, 1, 4, 3))
    xq = xq.reshape(B, 128, NG * GQ)
    xnat = x8.reshape(B, C, WXV * I)
    for kcore in range(N_CORES):
        for bo in OFFLOAD:
            xq[kcore * B_LOC + bo] = xnat[kcore * B_LOC + bo]

    # na_col[p=(c32,i4), (b, k, g)] = na[b, 32g+c32, 4k+i4] * prescale
    nacol = na.reshape(B, NG, CG, NK, IK).transpose(2, 4, 0, 3, 1)
    nacol = np.ascontiguousarray(nacol).reshape(128, B * NK * NG)
    nacol = (nacol * OUT_PRESCALE).astype(np.float32)

    # masks: mask64[p, j] = (j >= 32) & (p//4 == j-32); mask32[p, m] = (p//4 == m)
    p4 = np.arange(128) // IK
    m32 = (p4[:, None] == np.arange(CG)[None, :])
    mask = np.concatenate(
        [np.zeros((128, CG), bool), m32, m32], axis=1
    ).astype(np.float32)

    # na2[c, (b, i)] = na[b, c, i] * prescale (for the DVE slices)
    na2 = (na.transpose(1, 0, 2) * OUT_PRESCALE).astype(np.float32)

    in_maps = []
    for kcore in range(N_CORES):
        b0 = kcore * B_LOC
        nci = nacol.reshape(128, B, NK * NG)[:, b0 : b0 + B_LOC]
        ncif = np.ascontiguousarray(nci).reshape(128, -1).astype(np.float32)
        parts = [ncif.view(ml_dtypes.bfloat16),
                 mask.astype(ml_dtypes.bfloat16)]
        if OFFLOAD:
            na2c = np.concatenate(
                [na2[:, b0 + bo] for bo in OFFLOAD], axis=1)
            parts.append(np.ascontiguousarray(na2c)
                         .astype(ml_dtypes.bfloat16))
        cst = np.ascontiguousarray(np.concatenate(parts, axis=1))
        # custom image: [ cst bytes | slice0 g3 | slice0 g0-2 | x1..x15 ]
        ximg = np.concatenate(
            [cst.view(X_NP_DT),
             xq[b0][:, 3 * GQ : 4 * GQ], xq[b0][:, 0 : 3 * GQ]]
            + [xq[b0 + i] for i in range(1, B_LOC)], axis=1)
        in_maps.append({"x": np.ascontiguousarray(ximg)})
    return in_maps


def _gather(results) -> np.ndarray:
    # per-core out is [C, B_LOC, WXV] (c-major for store efficiency)
    out = np.concatenate(
        [np.asarray(r["out"]).transpose(1, 0, 2) for r in results], axis=0
    )
    out = out.astype(np.float32) * (1.0 / OUT_PRESCALE)
    return out.reshape(B, C, X, Y, Y)


def _run(inputs: dict, trace: bool = False, trace_cores=None):
    in_maps = _make_in_maps(inputs)
    nc = _get_compiled()
    res = run_bass_kernel_spmd(
        nc,
        in_maps,
        core_ids=list(range(N_CORES)),
        trace=trace,
        trace_cores=trace_cores,
    )
    return _gather(res.results), res


def kernel(**inputs) -> np.ndarray:
    out, _ = _run(inputs, trace=False)
    return out


# revision 37
# speedup vs baseline: 1.0973x; 1.0861x over previous
"""Trainium2 Bass kernel for nn_FeatureContraction.

Computes out[b,c,w,x,v] = sum_i x[b,c,w,x,v,i] * node_attributes[b,c,i]
with B=C=128, X=3, Y=16 (wxv = 3*16*16 = 768, i = 16).

Strategy (8 NeuronCores, data-parallel over b; PE does the math):
  - x is uploaded as fp8 e3m4 (4 mantissa bits), host-packed so each
    b-slice is one [128, 12288] image: partition p = (c32, i4) with
    c32 = c%32 within a 32-channel group, i4 = i%4 within an i-chunk;
    free axis = (g, k, w).
  - per (g, k): one matmul with a block-diagonal stationary
    S[(c32,i4), c32'] = delta * na[32g+c32, 4k+i4] and moving rhs
    x[(c32,i4), w]; the 4 i-chunks (k) accumulate in PSUM; output
    strip = psum partitions [32g, 32g+32).
    Group 3 (strip base 96) is inexpressible as an AP base partition
    (rust IR allows only 0/32/64), so it uses a [128, 64] stationary
    [0 | diag] at base 64 and is emitted FIRST: its start=True zeroes
    rows 64-95, which group 2's own start=True then overwrites.
  - stationaries are built ON DEVICE by the (otherwise idle) DVE:
    S = mask * na_col from a host-packed f32 table (one DMA with the
    masks); the 3 narrow groups of one k-chunk build in a single
    broadcast tensor_tensor.  The 0.25 output pre-scale is folded into
    the table on host (undone by *4 on host after gather).
  - whole contraction per b-slice = 32 matmuls into one [128, 768]
    f32 PSUM image; ACT/DVE cast it to fp8 e3m4 (output rides at
    1 B/elem; rel err ~1.9e-2 vs the 2e-2 gate).
  - DMA plan (all x on the sync HWDGE ring -- cross-ring DMAs are
    globally issue-serialized and sem-lane coupled, so a second ring
    does not help): consts first, slice 0 as g3 per-k chunks then
    quarters (fast PE ramp), middle slices as g3-quarter + one
    3-quarter DMA (PE consumes g3 first, so it starts each slice
    ~3 us before the slice's tail lands), slice 15 as quarters with
    g2 per-k chunks last; slice 3 is contracted on the DVE
    (broadcast multiply + segment reduce) to keep the PE strictly
    under the stream pace.  Output stores ride the scalar HWDGE
    ring 4 slices per store; slice 15 drains/stores per 32-row
    strip so only g2's strip trails the last x byte.  A handful of
    zero matmuls at body start warm the PE HAM clock gate.
  - HBM per core: 24 MiB x + 1.5 MiB out (both fp8) -> ~75 us at
    358 GB/s; PE moving-data ~76 us busy -> both near-floor.
"""

import os
import sys

for _p in ("/opt/trn_rl_repo",):
    if _p not in sys.path:
        sys.path.append(_p)

import ml_dtypes
import numpy as np

import concourse.bass as bass
import concourse.mybir as mybir
import concourse.tile as tile
from concourse import bacc
from concourse.bass_utils import run_bass_kernel_spmd

# Problem dims (hardcoded per spec)
B, C, X, Y = 128, 128, 3, 16
WXV = X * Y * Y          # 768
I = Y                    # 16 (contraction axis)
N_CORES = 8
B_LOC = B // N_CORES     # 16 b-slices per core

NG = 4                   # channel groups of 32 (PSUM col-strip aligned)
CG = C // NG             # 32 channels per group
NK = 4                   # i-chunks of 4: K = CG*4 = 128 partitions
IK = I // NK             # 4
W_H0 = 512               # h0 span: exactly one 2 KB f32 PSUM bank
W_H1 = WXV - W_H0        # 256: the last-drained half, kept small for the tail
GQ = NK * WXV            # 3072: one group's x columns per b-slice
SKB = 2 * CG + 3 * CG    # 160 stationary cols per (b, k): [g3w|g0|g1|g2]
OBATCH = 4               # b-slices per output store
NACW = B_LOC * NK * NG   # na table cols per core: (b, k, g) g-contiguous
MSKW = 3 * CG            # mask cols: [mask64 | mask32]

F32 = mybir.dt.float32
BF16 = mybir.dt.bfloat16
F8E3 = mybir.dt.float8e3

X_DT = os.environ.get("FC_X_DT", "f8e3")  # "f8e3" | "bf16" for A/B tests
X_MYBIR_DT = {"f8e3": F8E3, "bf16": BF16}[X_DT]
X_NP_DT = {"f8e3": ml_dtypes.float8_e3m4, "bf16": ml_dtypes.bfloat16}[X_DT]

OUT_DT = os.environ.get("FC_OUT_DT", "f8e3")  # "f8e3" | "bf16"
OUT_MYBIR_DT = {"f8e3": F8E3, "bf16": BF16}[OUT_DT]
OUT_NP_DT = {"f8e3": ml_dtypes.float8_e3m4, "bf16": ml_dtypes.bfloat16}[OUT_DT]
# pre-scale folded into the na table so fp8 outputs stay in e3m4 range
OUT_PRESCALE = 0.25 if OUT_DT == "f8e3" else 1.0

GORDER = (3, 0, 1, 2)    # wide group 3 first (see module docstring)
N_WARM = 5               # zero matmuls to lift the PE HAM clock gate
OFFLOAD = (4,)           # DVE slice: early x arrival, but in batch 4-7
                         # whose store is naturally late (the DVE chain
                         # ends ~56us; batch 0-3 must not wait on it)
NA2W = len(OFFLOAD) * I  # na2 table cols for the DVE slices only

_COMPILED = None


def _build():
    nc = bacc.Bacc("TRN2", target_bir_lowering=False, debug=False,
                   num_devices=N_CORES)

    # one input image per core: [ cst | slice0 g3k0 | slice0 g3k1-3 |
    # slice0 g0-2 | slices 1..15 ] so the stream's FIRST dma carries
    # both the consts and the first PE chunk in a single issue slot.
    # cst bytes: nacol as f32, masks/na2 as bf16, all viewed as fp8.
    CSTF8 = 4 * NACW + 2 * (MSKW + NA2W)
    SL0 = CSTF8 + NG * GQ
    x_d = nc.dram_tensor("x", [128, SL0 + (B_LOC - 1) * NG * GQ],
                         X_MYBIR_DT, kind="ExternalInput")
    # c-major so a multi-slice store has one contiguous line per partition
    out_d = nc.dram_tensor("out", [C, B_LOC, WXV], OUT_MYBIR_DT,
                           kind="ExternalOutput")

    with tile.TileContext(nc) as tc:
        with (
            tc.tile_pool(name="const", bufs=1) as constp,
            tc.tile_pool(name="xp", bufs=5) as xp,
            tc.tile_pool(name="xn", bufs=1) as xnp,
            tc.tile_pool(name="xq", bufs=6) as xqp,
            tc.tile_pool(name="sp", bufs=B_LOC - len(OFFLOAD)) as sp,
            tc.tile_pool(name="yp", bufs=1) as yp,
            tc.tile_pool(name="yr", bufs=1) as yrp,
            tc.tile_pool(name="outp", bufs=3) as outp,
            tc.tile_pool(name="psp", bufs=4, space="PSUM") as psp,
        ):
            # head tile = consts + slice0's g3k0 chunk (one DMA)
            hd = constp.tile([128, CSTF8 + WXV], X_MYBIR_DT)
            nacol = hd[:, : 4 * NACW].bitcast(F32)
            m64 = hd[:, 4 * NACW : 4 * NACW + 4 * CG].bitcast(BF16)
            m32 = hd[:, 4 * NACW + 4 * CG : 4 * NACW + 6 * CG].bitcast(BF16)
            na2 = hd[:, 4 * NACW + 6 * CG : CSTF8].bitcast(BF16)

            # PE warm-up: zero matmuls on a memset tile lift the HAM
            # clock gate (1.2 -> 2.4 GHz takes ~3.4 us of PE activity)
            # while the first x tile is still in flight
            wt = constp.tile([128, 32 + W_H0], BF16)
            nc.gpsimd.memset(wt[:], 0)
            dps = psp.tile([128, W_H0], F32, tag="ps0")
            for _ in range(N_WARM):
                nc.tensor.matmul(dps[:32, :], wt[:, :32], wt[:, 32:],
                                 start=True, stop=True)

            # x load schedule, all on the sync HWDGE FIFO, emitted up
            # front: small chunks at the ramp (PE starts on 98 KB), one
            # 1.57 MB line-rate DMA per slice in the steady state,
            # chunks again at the tail so the last bytes are consumed
            # on arrival.  Buffer recycling paces the later issues.
            xloads = {}

            def xoff(b, c0):
                # slice b's column c0 in the custom DRAM layout
                assert b >= 1
                return SL0 + (b - 1) * NG * GQ + c0

            def load_chunks(b, g):
                xck = []
                for kk in range(NK):
                    xc = xqp.tile([128, WXV], X_MYBIR_DT, tag="xk")
                    nc.sync.dma_start(
                        xc[:],
                        x_d[:, xoff(b, g * GQ + kk * WXV) :
                            xoff(b, g * GQ + (kk + 1) * WXV)])
                    xck.append(xc)
                xloads.setdefault(b, {})[g] = xck

            def load_quarter(b, g):
                xt = xqp.tile([128, GQ], X_MYBIR_DT, tag="x4")
                nc.sync.dma_start(
                    xt[:], x_d[:, xoff(b, g * GQ) : xoff(b, (g + 1) * GQ)])
                xloads.setdefault(b, {})[g] = xt

            def load_full(b):
                if b in OFFLOAD:
                    xf = xnp.tile([128, NG * GQ], X_MYBIR_DT, tag="x")
                    nc.sync.dma_start(
                        xf[:], x_d[:, xoff(b, 0) : xoff(b, NG * GQ)])
                    xloads.setdefault(b, {})["nat"] = xf
                    return
                # g3's quarter rides FIRST (PE consumes g3 first), the
                # other three follow in one DMA: PE starts each slice
                # ~3 us earlier than with a single whole-slice DMA
                xa = xp.tile([128, GQ], X_MYBIR_DT, tag="xa")
                nc.sync.dma_start(
                    xa[:], x_d[:, xoff(b, 3 * GQ) : xoff(b, 4 * GQ)])
                xr = xp.tile([128, 3 * GQ], X_MYBIR_DT, tag="xr")
                nc.sync.dma_start(xr[:], x_d[:, xoff(b, 0) : xoff(b, 3 * GQ)])
                xloads.setdefault(b, {})[3] = xa
                for g in range(3):
                    xloads[b][g] = xr[:, g * GQ : (g + 1) * GQ]

            # head DMA: consts + slice0 g3k0 in ONE issue slot leads
            # the sync FIFO; g3 k1-3 chunks and the quarters follow
            # (custom DRAM layout puts them contiguously after cst)
            nc.sync.dma_start(hd[:], x_d[:, : CSTF8 + WXV])
            xck0 = [hd[:, CSTF8:]]
            for kk in range(1, NK):
                xc = xqp.tile([128, WXV], X_MYBIR_DT, tag="xk")
                nc.sync.dma_start(
                    xc[:], x_d[:, CSTF8 + kk * WXV : CSTF8 + (kk + 1) * WXV])
                xck0.append(xc)
            xloads[0] = {3: xck0}
            for g in (0, 1, 2):        # slice 0 quarters
                xt = xqp.tile([128, GQ], X_MYBIR_DT, tag="x4")
                nc.sync.dma_start(
                    xt[:], x_d[:, CSTF8 + NK * WXV + g * GQ :
                               CSTF8 + NK * WXV + (g + 1) * GQ])
                xloads[0][g] = xt
            for b in range(1, B_LOC - 1):
                load_full(b)
            # slice 15: quarters g3,g0,g1 (each strip's matmuls and
            # drains start on its own arrival), g2 per-k chunks last
            # (consumed on arrival at the stream tail)
            load_quarter(15, 3)
            load_quarter(15, 0)
            load_quarter(15, 1)
            load_chunks(15, 2)

            # all stationaries build upfront on DVE (they only need the
            # const tile); DVE's later offload-slice work then never
            # blocks a build the PE is waiting for
            sts = {}
            for b in range(B_LOC):
                if b in OFFLOAD:
                    continue
                st = sp.tile([128, NK * SKB], BF16, tag="s")
                sts[b] = st
                for k in range(NK):
                    j = (b * NK + k) * NG
                    nc.vector.tensor_scalar_mul(
                        st[:, k * SKB : k * SKB + 2 * CG],
                        m64, nacol[:, j + 3 : j + 4])
                    nc.vector.tensor_tensor(
                        st[:, k * SKB + 2 * CG : (k + 1) * SKB]
                        .rearrange("p (g c) -> p g c", g=3),
                        nacol[:, j : j + 3].unsqueeze(-1)
                        .broadcast_to([128, 3, CG]),
                        m32.unsqueeze(1).broadcast_to([128, 3, CG]),
                        mybir.AluOpType.mult)

            ot = None
            ota = None
            for b in range(B_LOC):
                xts = xloads.pop(b)
                first_b = b == 0
                last_b = b == B_LOC - 1

                if b in OFFLOAD:
                    # DVE path: x for this slice is packed [c, (w, i)]
                    # (natural layout); broadcast-multiply by na2[c, i]
                    # then segment-reduce the 16 i's; ACT casts to fp8
                    xn = xts["nat"]
                    y = yp.tile([128, NG * GQ], BF16, tag="y")
                    nc.vector.tensor_tensor(
                        y[:].rearrange("p (w i) -> p w i", i=I),
                        xn[:].rearrange("p (w i) -> p w i", i=I),
                        na2[:, OFFLOAD.index(b) * I :
                            (OFFLOAD.index(b) + 1) * I].unsqueeze(1)
                        .broadcast_to([128, WXV, I]),
                        mybir.AluOpType.mult)
                    red = yrp.tile([128, WXV], F32, tag="r")
                    nc.vector.tensor_reduce(
                        red[:], y[:].rearrange("p (w i) -> p w i", i=I),
                        mybir.AxisListType.X, mybir.AluOpType.add)
                    ob = b % OBATCH
                    if ob == 0:
                        ot = outp.tile([C, OBATCH * WXV], OUT_MYBIR_DT,
                                       tag="out")
                    o0 = ob * WXV
                    nc.scalar.copy(ot[:, o0 : o0 + WXV], red[:])
                    if ob == OBATCH - 1:
                        b0 = b - (OBATCH - 1)
                        nc.scalar.dma_start(out_d[:, b0 : b0 + OBATCH, :],
                                            ot[:])
                    continue

                st = sts[b]
                ps0 = psp.tile([128, W_H0], F32, tag="ps0")
                ps1 = psp.tile([128, W_H1], F32, tag="ps1")
                ps = {0: ps0, 1: ps1}
                hspan = {0: (0, W_H0), 1: (W_H0, WXV)}
                otb = None
                if last_b:
                    otb = outp.tile([C, WXV], OUT_MYBIR_DT, tag="outb")
                for g in GORDER:
                    if last_b and g == 2:
                        # strips g0+g1 (psum rows 0:64) and g3 (96:128)
                        # are complete: drain + store them NOW, before
                        # g2's matmuls are emitted, so their waits don't
                        # inherit g2's later ticks and they fly while
                        # the stream tail lands
                        nc.scalar.copy(otb[0:64, :W_H0], ps[0][0:64, :])
                        nc.vector.tensor_copy(otb[0:64, W_H0:],
                                              ps[1][0:64, :])
                        nc.scalar.copy(otb[96:128, :W_H0],
                                       ps[0][96:128, :])
                        nc.vector.tensor_copy(otb[96:128, W_H0:],
                                              ps[1][96:128, :])
                        nc.sync.dma_start(out_d[0:64, b : b + 1, :],
                                          otb[0:64, :])
                        nc.sync.dma_start(out_d[96:128, b : b + 1, :],
                                          otb[96:128, :])
                    if (first_b and g == 3) or (last_b and g == 2):
                        # chunked tail/head group: k-inner per w-region
                        # (j outer) so each PSUM bank has at most ONE
                        # open accumulation group at a time (interleaved
                        # groups within a bank lose accumulations on HW)
                        nck = len(xts[g])
                        kh = nck // NK  # chunks per k
                        for j in range(kh):
                            c0 = j * (WXV // kh)
                            for k in range(NK):
                                if g == 3:
                                    lhsT = st[:, k * SKB : k * SKB + 2 * CG]
                                    oap = {h: ps[h][2 * CG : 4 * CG, :]
                                           for h in range(2)}
                                else:
                                    s0 = k * SKB + 2 * CG + g * CG
                                    lhsT = st[:, s0 : s0 + CG]
                                    oap = {h: ps[h][CG * g : CG * (g + 1), :]
                                           for h in range(2)}
                                for h in range(2):
                                    w0, w1 = hspan[h]
                                    lo = max(w0, c0)
                                    hi = min(w1, c0 + WXV // kh)
                                    if lo >= hi:
                                        continue
                                    nc.tensor.matmul(
                                        oap[h][:, lo - w0 : hi - w0],
                                        lhsT,
                                        xts[g][k * kh + j][:, lo - c0 : hi - c0],
                                        start=(k == 0),
                                        stop=(k == NK - 1),
                                    )
                        continue
                    for h in range(2):
                        w0, w1 = hspan[h]
                        for k in range(NK):
                            if g == 3:
                                lhsT = st[:, k * SKB : k * SKB + 2 * CG]
                                oap = ps[h][2 * CG : 4 * CG, :]
                            else:
                                s0 = k * SKB + 2 * CG + g * CG
                                lhsT = st[:, s0 : s0 + CG]
                                oap = ps[h][CG * g : CG * (g + 1), :]
                            nc.tensor.matmul(
                                oap,
                                lhsT,
                                xts[g][:, k * WXV + w0 : k * WXV + w1],
                                start=(k == 0),
                                stop=(k == NK - 1),
                            )

                # ACT drains both PSUM halves (DVE is busy with the
                # offload slices); stores ride the scalar HWDGE ring
                # except the last, which takes the (drained) sync ring
                if last_b:
                    # only g2's strip (64:96, fed by the last chunks)
                    # rides the critical tail; the rest stored above.
                    # both halves drain on DVE: serial ~0.5us beats
                    # ACT's ~0.66us fixed-cost copy on this path
                    nc.vector.tensor_copy(otb[64:96, :W_H0],
                                          ps[0][64:96, :])
                    nc.vector.tensor_copy(otb[64:96, W_H0:],
                                          ps[1][64:96, :])
                    nc.sync.dma_start(out_d[64:96, b : b + 1, :],
                                      otb[64:96, :])
                elif b >= 12:
                    ob = b - 12
                    if ob == 0:
                        ota = outp.tile([C, 3 * WXV], OUT_MYBIR_DT, tag="outa")
                    o0 = ob * WXV
                    nc.scalar.copy(ota[:, o0 : o0 + W_H0], ps[0][:])
                    nc.scalar.copy(ota[:, o0 + W_H0 : o0 + WXV], ps[1][:])
                    if ob == 2:
                        nc.scalar.dma_start(out_d[:, 12:15, :], ota[:])
                else:
                    ob = b % OBATCH
                    if ob == 0:
                        ot = outp.tile([C, OBATCH * WXV], OUT_MYBIR_DT,
                                       tag="out")
                    o0 = ob * WXV
                    nc.scalar.copy(ot[:, o0 : o0 + W_H0], ps[0][:])
                    nc.scalar.copy(ot[:, o0 + W_H0 : o0 + WXV], ps[1][:])
                    if ob == OBATCH - 1:
                        b0 = b - (OBATCH - 1)
                        nc.scalar.dma_start(out_d[:, b0 : b0 + OBATCH, :],
                                            ot[:])

    nc.compile()
    return nc


def _get_compiled():
    global _COMPILED
    if _COMPILED is None:
        _COMPILED = _build()
    return _COMPILED


def _make_in_maps(inputs: dict):
    x = np.asarray(inputs["x"], dtype=np.float32)
    na = np.asarray(inputs["node_attributes"], dtype=np.float32)

    # x[b, c, w, i] -> xq[b, p=(c32,i4), (g, k), w], cast first (cheaper
    # to transpose 1-2 B elems than 4 B); the DVE-offload slices keep
    # the natural [c, (w, i)] layout instead
    x8 = x.reshape(B, C, WXV, I).astype(X_NP_DT)
    xq = x8.reshape(B, NG, CG, WXV, NK, IK)
    xq = np.ascontiguousarray(xq.transpose(0, 2, 5, 1, 4, 3))
    xq = xq.reshape(B, 128, NG * GQ)
    xnat = x8.reshape(B, C, WXV * I)
    for kcore in range(N_CORES):
        for bo in OFFLOAD:
            xq[kcore * B_LOC + bo] = xnat[kcore * B_LOC + bo]

    # na_col[p=(c32,i4), (b, k, g)] = na[b, 32g+c32, 4k+i4] * prescale
    nacol = na.reshape(B, NG, CG, NK, IK).transpose(2, 4, 0, 3, 1)
    nacol = np.ascontiguousarray(nacol).reshape(128, B * NK * NG)
    nacol = (nacol * OUT_PRESCALE).astype(np.float32)

    # masks: mask64[p, j] = (j >= 32) & (p//4 == j-32); mask32[p, m] = (p//4 == m)
    p4 = np.arange(128) // IK
    m32 = (p4[:, None] == np.arange(CG)[None, :])
    mask = np.concatenate(
        [np.zeros((128, CG), bool), m32, m32], axis=1
    ).astype(np.float32)

    # na2[c, (b, i)] = na[b, c, i] * prescale (for the DVE slices)
    na2 = (na.transpose(1, 0, 2) * OUT_PRESCALE).astype(np.float32)

    in_maps = []
    for kcore in range(N_CORES):
        b0 = kcore * B_LOC
        nci = nacol.reshape(128, B, NK * NG)[:, b0 : b0 + B_LOC]
        ncif = np.ascontiguousarray(nci).reshape(128, -1).astype(np.float32)
        parts = [ncif.view(ml_dtypes.bfloat16),
                 mask.astype(ml_dtypes.bfloat16)]
        if OFFLOAD:
            na2c = np.concatenate(
                [na2[:, b0 + bo] for bo in OFFLOAD], axis=1)
            parts.append(np.ascontiguousarray(na2c)
                         .astype(ml_dtypes.bfloat16))
        cst = np.ascontiguousarray(np.concatenate(parts, axis=1))
        # custom image: [ cst bytes | slice0 g3 | slice0 g0-2 | x1..x15 ]
        ximg = np.concatenate(
            [cst.view(X_NP_DT),
             xq[b0][:, 3 * GQ : 4 * GQ], xq[b0][:, 0 : 3 * GQ]]
            + [xq[b0 + i] for i in range(1, B_LOC)], axis=1)
        in_maps.append({"x": np.ascontiguousarray(ximg)})
    return in_maps


def _gather(results) -> np.ndarray:
    # per-core out is [C, B_LOC, WXV] (c-major for store efficiency)
    out = np.concatenate(
        [np.asarray(r["out"]).transpose(1, 0, 2) for r in results], axis=0
    )
    out = out.astype(np.float32) * (1.0 / OUT_PRESCALE)
    return out.reshape(B, C, X, Y, Y)


def _run(inputs: dict, trace: bool = False, trace_cores=None):
    in_maps = _make_in_maps(inputs)
    nc = _get_compiled()
    res = run_bass_kernel_spmd(
        nc,
        in_maps,
        core_ids=list(range(N_CORES)),
        trace=trace,
        trace_cores=trace_cores,
    )
    return _gather(res.results), res


def kernel(**inputs) -> np.ndarray:
    out, _ = _run(inputs, trace=False)
    return out
